# revision 9
# baseline (speedup 1.0000x reference)
"""Trainium2 Bass kernel for nn_CorrClassLoss.

Reference computation (B=4, C=19, H=512, W=1024, N=5000, IGNORE=255):
  ref_class = argmax_c inputs_ref[b].reshape(C, H*W)      # flat W-major
  lin_ref   = 512*y_ref + x_ref    (NOTE: linearized with H, kept faithfully)
  lin_other = 512*y_other + x_other
  gathered  = ref_class[b, lin_ref]
  target[b, lin_other] = gathered  (scatter, last write wins; rest IGNORE)
  loss = mean over non-ignored pixels of -log_softmax(inputs_other)[b, target, px]

Since lin = 512*y + x with x,y in [0,512), only flat positions [0, 262144)
are ever touched, and at most N unique scatter destinations per batch
contribute to the loss:

  loss = -(1/cnt) * sum over unique dests d (last writer j, src s_j) of
         [ x_other[b, cls(s_j), d] - ln(sum_c exp(x_other[b, c, d])) ]
  cls(s) = argmax_c x_ref[b, c, s],  cnt = total unique dests.

Strategy (8 cores, data-parallel over (batch, half-of-correspondences)):
  Host does index-only math (dedup last-wins, split j by the pixel-half of
  s_j, pack padded gather-offset tables) and hands each core a single
  pixel-major fp16 tensor cat_t = [ref_half_t; other_t; zero-row] (a
  layout/sharding choice; all value compute happens on device).
  Device per core: ONE indirect gather (multi-column offset table read
  straight from DRAM) fetches the ref vector at s_j and the other vector
  at d_j for every correspondence; pad slots point at the zero row so no
  masking/memset is needed.  Argmax one-hot via grouped max + is_ge;
  t1 = onehot . other_vec;  t2 = ln(sum_c exp(other_vec[c])).
  Output [P, 1] = per-partition sums of (t1 - t2); host sums partitions,
  adds back the pads' exactly-known -ln(19) contribution, and divides.
"""

import sys

if "/opt/trn_rl_repo" not in sys.path:
    sys.path.insert(0, "/opt/trn_rl_repo")

import numpy as np

B, C, H, W = 4, 19, 512, 1024
HW = H * W                 # 524288
NPIX = 262144              # touched flat range [0, 262144)
NPIX_H = NPIX // 2         # 131072 source pixels per core
N = 5000
NCORES = 8

P = 128                    # partitions
M = NPIX_H + NPIX + 1      # cat_t rows: ref half + other + one zero row
ZERO_ROW = NPIX_H + NPIX          # row index of the zero row

_programs = {}


def _build_program(cgg):
    import concourse.bass as bass
    import concourse.bacc as bacc
    import concourse.mybir as mybir
    import concourse.tile as tile

    GW = cgg * 19

    nc = bacc.Bacc("TRN2", target_bir_lowering=False, debug=False,
                   num_devices=NCORES)

    # fp16 pixel-major shards: [ref half (NPIX_H); other (NPIX); zeros (1)]
    cat_t = nc.dram_tensor("cat_t", [M, C], mybir.dt.float16,
                           kind="ExternalInput")
    # gather offsets (row indices into cat_t): cols [0,cgg) = s_local,
    # cols [cgg,2cgg) = NPIX_H+d; element j at [j%P, j//P]; pads -> ZERO_ROW
    offs = nc.dram_tensor("offs", [P, 2 * cgg], mybir.dt.int32,
                          kind="ExternalInput")
    out = nc.dram_tensor("out", [P, 1], mybir.dt.float32,
                         kind="ExternalOutput")

    cat_flat = cat_t.rearrange("p c -> (p c)")

    with tile.TileContext(nc) as tc:
        with tc.tile_pool(name="gb", bufs=1) as gb:
            # offset table must live in SBUF for the HW descriptor generator
            so = gb.tile([P, 2 * cgg], mybir.dt.int32)
            nc.sync.dma_start(out=so[:], in_=offs[:, :])
            # one gather for everything: ref vectors land in G[:, :GW],
            # other vectors in G[:, GW:]; pad slots read the zero row.
            # in_ is the flat 1D view (one contiguous run) so each
            # partition's 2*GW-element row is one modeled descriptor.
            G = gb.tile([P, 2 * GW], mybir.dt.float16)
            nc.gpsimd.indirect_dma_start(
                out=G[:],
                out_offset=None,
                in_=cat_flat[None, :],
                in_offset=bass.IndirectOffsetOnAxis(ap=so[:, :], axis=1),
                bounds_check=M * 19 - 1,
                oob_is_err=False,
            )

            Rv = G[:, 0:GW].rearrange("p (g c) -> p g c", c=19)
            R2 = G[:, GW:2 * GW]
            m2 = gb.tile([P, cgg], mybir.dt.float16)
            nc.vector.tensor_reduce(out=m2[:], in_=Rv,
                                    axis=mybir.AxisListType.X,
                                    op=mybir.AluOpType.max)
            eq = gb.tile([P, GW], mybir.dt.float16)
            eqv = eq[:].rearrange("p (g c) -> p g c", c=19)
            nc.vector.tensor_tensor(
                out=eqv, in0=Rv,
                in1=m2[:, :, None].to_broadcast([P, cgg, 19]),
                op=mybir.AluOpType.is_ge,
            )
            nc.vector.tensor_tensor(out=eq[:], in0=eq[:], in1=R2,
                                    op=mybir.AluOpType.mult)
            t1 = gb.tile([P, cgg], mybir.dt.float32)
            nc.vector.tensor_reduce(out=t1[:], in_=eqv,
                                    axis=mybir.AxisListType.X,
                                    op=mybir.AluOpType.add)

            e2 = gb.tile([P, GW], mybir.dt.float32)
            nc.scalar.activation(e2[:], R2,
                                 mybir.ActivationFunctionType.Exp)
            S2 = gb.tile([P, cgg], mybir.dt.float32)
            nc.vector.tensor_reduce(
                out=S2[:],
                in_=e2[:].rearrange("p (g c) -> p g c", c=19),
                axis=mybir.AxisListType.X, op=mybir.AluOpType.add)
            L2 = gb.tile([P, cgg], mybir.dt.float32)
            nc.scalar.activation(L2[:], S2[:],
                                 mybir.ActivationFunctionType.Ln)

            nc.vector.tensor_tensor(out=t1[:], in0=t1[:], in1=L2[:],
                                    op=mybir.AluOpType.subtract)
            vr = gb.tile([P, 1], mybir.dt.float32)
            nc.vector.tensor_reduce(out=vr[:], in_=t1[:],
                                    axis=mybir.AxisListType.X,
                                    op=mybir.AluOpType.add)
            nc.sync.dma_start(out=out[:, :], in_=vr[:])

    nc.finalize()
    return nc


def _get_program(cgg):
    if cgg not in _programs:
        _programs[cgg] = _build_program(cgg)
    return _programs[cgg]


def _host_prep(inds_ref, inds_other):
    """Index-only host math: dedup scatter (last wins), partition per core."""
    ir = np.asarray(inds_ref).astype(np.int64)      # [B, 2, N]
    io = np.asarray(inds_other).astype(np.int64)
    valid = ((ir[:, 0] >= 0) & (ir[:, 0] < W) & (ir[:, 1] >= 0) & (ir[:, 1] < H)
             & (io[:, 0] >= 0) & (io[:, 0] < W) & (io[:, 1] >= 0)
             & (io[:, 1] < H))                       # [B, N]
    lin_ref = H * ir[:, 1] + ir[:, 0]                # [B, N]
    lin_other = H * io[:, 1] + io[:, 0]

    per_core = []
    count = 0
    for b in range(B):
        v = valid[b]
        lo = lin_other[b][v]
        lr = np.clip(lin_ref[b][v], 0, HW - 1)
        # last-write-wins dedup on destinations
        u, first_rev = np.unique(lo[::-1], return_index=True)
        last_idx = len(lo) - 1 - first_rev
        d_arr = u.astype(np.int64)
        s_arr = lr[last_idx].astype(np.int64)
        count += len(u)
        for h in range(2):
            sel = (s_arr // NPIX_H) == h
            s_local = s_arr[sel] - h * NPIX_H
            d_sel = d_arr[sel]
            per_core.append({
                "b": b, "h": h,
                "s": s_local, "d": d_sel,
            })
    return per_core, count


def _pack_offs(pc, cgg):
    offs = np.full((P, 2 * cgg), ZERO_ROW * 19, dtype=np.int32)
    s, d = pc["s"], pc["d"]
    n = len(s)
    assert n <= cgg * P
    jj = np.arange(n)
    offs[jj % P, jj // P] = s * 19
    offs[jj % P, cgg + jj // P] = (NPIX_H + d) * 19
    return offs


def _make_in_maps(inputs_ref, inputs_other, per_core, cgg):
    ref_flat = inputs_ref.reshape(B, C, HW)
    other_flat = inputs_other.reshape(B, C, HW)
    other_cache = {}
    zrow = np.zeros((1, C), dtype=np.float16)
    in_maps = []
    for pc in per_core:
        b, h = pc["b"], pc["h"]
        ref_td = np.ascontiguousarray(
            ref_flat[b, :, h * NPIX_H:(h + 1) * NPIX_H].T).astype(np.float16)
        if b not in other_cache:
            other_cache[b] = np.ascontiguousarray(
                other_flat[b, :, :NPIX].T).astype(np.float16)
        cat = np.concatenate([ref_td, other_cache[b], zrow], axis=0)
        in_maps.append({
            "cat_t": cat,
            "offs": _pack_offs(pc, cgg),
        })
    return in_maps


def kernel(inputs_ref, inputs_other, inds_ref, inds_other, weights):
    from concourse.bass_utils import run_bass_kernel_spmd

    inputs_ref = np.asarray(inputs_ref, dtype=np.float32)
    inputs_other = np.asarray(inputs_other, dtype=np.float32)

    per_core, count = _host_prep(inds_ref, inds_other)
    # exact-fit capacity: compile (and cache) the program for the actual
    # worst-core correspondence count, rounded up to whole 128-columns
    max_n = max(len(pc["s"]) for pc in per_core)
    cgg = max(1, -(-max_n // P))
    nc = _get_program(cgg)

    in_maps = _make_in_maps(inputs_ref, inputs_other, per_core, cgg)
    res = run_bass_kernel_spmd(nc, in_maps, core_ids=list(range(NCORES)))
    total = 0.0
    ln19 = float(np.log(np.float32(19.0)))
    for pc, r in zip(per_core, res.results):
        o = np.asarray(r["out"], dtype=np.float64)
        n_pad = cgg * P - len(pc["s"])
        total += o.sum() + n_pad * ln19
    loss = -total / max(count, 1)
    return np.float32(loss)


# revision 13
# speedup vs baseline: 1.0146x; 1.0146x over previous
"""Trainium2 Bass kernel for nn_CorrClassLoss.

Reference computation (B=4, C=19, H=512, W=1024, N=5000, IGNORE=255):
  ref_class = argmax_c inputs_ref[b].reshape(C, H*W)      # flat W-major
  lin_ref   = 512*y_ref + x_ref    (NOTE: linearized with H, kept faithfully)
  lin_other = 512*y_other + x_other
  gathered  = ref_class[b, lin_ref]
  target[b, lin_other] = gathered  (scatter, last write wins; rest IGNORE)
  loss = mean over non-ignored pixels of -log_softmax(inputs_other)[b, target, px]

Since lin = 512*y + x with x,y in [0,512), only flat positions [0, 262144)
are ever touched, and at most N unique scatter destinations per batch
contribute to the loss:

  loss = -(1/cnt) * sum over unique dests d (last writer j, src s_j) of
         [ x_other[b, cls(s_j), d] - ln(sum_c exp(x_other[b, c, d])) ]
  cls(s) = argmax_c x_ref[b, c, s],  cnt = total unique dests.

Strategy (8 cores, data-parallel over (batch, half-of-correspondences)):
  Host does index-only math (dedup last-wins, split j by the pixel-half of
  s_j, pack padded gather-offset tables) and hands each core a single
  pixel-major fp16 tensor cat_t = [ref_half_t; other_t; zero-row] (a
  layout/sharding choice; all value compute happens on device).
  Device per core: ONE indirect gather (multi-column offset table read
  straight from DRAM) fetches the ref vector at s_j and the other vector
  at d_j for every correspondence; pad slots point at the zero row so no
  masking/memset is needed.  Argmax one-hot via grouped max + is_ge;
  t1 = onehot . other_vec;  t2 = ln(sum_c exp(other_vec[c])).
  Output [P, 1] = per-partition sums of (t1 - t2); host sums partitions,
  adds back the pads' exactly-known -ln(19) contribution, and divides.
"""

import sys

if "/opt/trn_rl_repo" not in sys.path:
    sys.path.insert(0, "/opt/trn_rl_repo")

import numpy as np

B, C, H, W = 4, 19, 512, 1024
HW = H * W                 # 524288
NPIX = 262144              # touched flat range [0, 262144)
NPIX_H = NPIX // 2         # 131072 source pixels per core
N = 5000
NCORES = 8

P = 128                    # partitions
M = NPIX_H + NPIX + 1      # cat_t rows: ref half + other + one zero row
ZERO_ROW = NPIX_H + NPIX          # row index of the zero row

_programs = {}


def _build_program(cgg):
    import concourse.bass as bass
    import concourse.bacc as bacc
    import concourse.mybir as mybir
    import concourse.tile as tile

    GW = cgg * 19

    nc = bacc.Bacc("TRN2", target_bir_lowering=False, debug=False,
                   num_devices=NCORES)

    # fp16 pixel-major shards: [ref half (NPIX_H); other (NPIX); zeros (1)]
    cat_t = nc.dram_tensor("cat_t", [M, C], mybir.dt.float16,
                           kind="ExternalInput")
    # gather offsets (row indices into cat_t): cols [0,cgg) = s_local,
    # cols [cgg,2cgg) = NPIX_H+d; element j at [j%P, j//P]; pads -> ZERO_ROW
    offs = nc.dram_tensor("offs", [P, 2 * cgg], mybir.dt.int32,
                          kind="ExternalInput")
    out = nc.dram_tensor("out", [P, cgg], mybir.dt.float32,
                         kind="ExternalOutput")

    cat_flat = cat_t.rearrange("p c -> (p c)")

    with tile.TileContext(nc) as tc:
        with (
            tc.tile_pool(name="gb", bufs=1) as gb,
            nc.allow_low_precision(
                reason="fp16 group sums of <=19 values; loss tolerance 2e-2"),
        ):
            # offset table must live in SBUF for the HW descriptor generator
            so = gb.tile([P, 2 * cgg], mybir.dt.int32)
            nc.sync.dma_start(out=so[:], in_=offs[:, :])
            # one gather for everything: ref vectors land in G[:, :GW],
            # other vectors in G[:, GW:]; pad slots read the zero row.
            # in_ is the flat 1D view (one contiguous run) so each
            # partition's 2*GW-element row is one modeled descriptor.
            G = gb.tile([P, 2 * GW], mybir.dt.float16)
            nc.gpsimd.indirect_dma_start(
                out=G[:],
                out_offset=None,
                in_=cat_flat[None, :],
                in_offset=bass.IndirectOffsetOnAxis(ap=so[:, :], axis=1),
                bounds_check=M * 19 - 1,
                oob_is_err=False,
            )

            Rv = G[:, 0:GW].rearrange("p (g c) -> p g c", c=19)
            R2 = G[:, GW:2 * GW]
            m2 = gb.tile([P, cgg], mybir.dt.float16)
            nc.vector.tensor_reduce(out=m2[:], in_=Rv,
                                    axis=mybir.AxisListType.X,
                                    op=mybir.AluOpType.max)
            eq = gb.tile([P, GW], mybir.dt.float16)
            eqv = eq[:].rearrange("p (g c) -> p g c", c=19)
            nc.vector.tensor_tensor(
                out=eqv, in0=Rv,
                in1=m2[:, :, None].to_broadcast([P, cgg, 19]),
                op=mybir.AluOpType.is_ge,
            )
            nc.vector.tensor_tensor(out=eq[:], in0=eq[:], in1=R2,
                                    op=mybir.AluOpType.mult)

            e2 = gb.tile([P, GW], mybir.dt.float16)
            nc.scalar.activation(e2[:], R2,
                                 mybir.ActivationFunctionType.Exp)
            # S2 scheduled before t1: the final subtract is gated by
            # L2 = Ln(S2), so get S2 off the DVE queue first
            S2 = gb.tile([P, cgg], mybir.dt.float16)
            nc.vector.tensor_reduce(
                out=S2[:],
                in_=e2[:].rearrange("p (g c) -> p g c", c=19),
                axis=mybir.AxisListType.X, op=mybir.AluOpType.add)
            L2 = gb.tile([P, cgg], mybir.dt.float32)
            nc.scalar.activation(L2[:], S2[:],
                                 mybir.ActivationFunctionType.Ln)

            t1 = gb.tile([P, cgg], mybir.dt.float16)
            nc.vector.tensor_reduce(out=t1[:], in_=eqv,
                                    axis=mybir.AxisListType.X,
                                    op=mybir.AluOpType.add)

            res = gb.tile([P, cgg], mybir.dt.float32)
            nc.vector.tensor_tensor(out=res[:], in0=t1[:], in1=L2[:],
                                    op=mybir.AluOpType.subtract)
            nc.sync.dma_start(out=out[:, :], in_=res[:])

    nc.finalize()
    return nc


def _get_program(cgg):
    if cgg not in _programs:
        _programs[cgg] = _build_program(cgg)
    return _programs[cgg]


def _host_prep(inds_ref, inds_other):
    """Index-only host math: dedup scatter (last wins), partition per core."""
    ir = np.asarray(inds_ref).astype(np.int64)      # [B, 2, N]
    io = np.asarray(inds_other).astype(np.int64)
    valid = ((ir[:, 0] >= 0) & (ir[:, 0] < W) & (ir[:, 1] >= 0) & (ir[:, 1] < H)
             & (io[:, 0] >= 0) & (io[:, 0] < W) & (io[:, 1] >= 0)
             & (io[:, 1] < H))                       # [B, N]
    lin_ref = H * ir[:, 1] + ir[:, 0]                # [B, N]
    lin_other = H * io[:, 1] + io[:, 0]

    per_core = []
    count = 0
    for b in range(B):
        v = valid[b]
        lo = lin_other[b][v]
        lr = np.clip(lin_ref[b][v], 0, HW - 1)
        # last-write-wins dedup on destinations
        u, first_rev = np.unique(lo[::-1], return_index=True)
        last_idx = len(lo) - 1 - first_rev
        d_arr = u.astype(np.int64)
        s_arr = lr[last_idx].astype(np.int64)
        count += len(u)
        for h in range(2):
            sel = (s_arr // NPIX_H) == h
            s_local = s_arr[sel] - h * NPIX_H
            d_sel = d_arr[sel]
            per_core.append({
                "b": b, "h": h,
                "s": s_local, "d": d_sel,
            })
    return per_core, count


def _pack_offs(pc, cgg):
    offs = np.full((P, 2 * cgg), ZERO_ROW * 19, dtype=np.int32)
    s, d = pc["s"], pc["d"]
    n = len(s)
    assert n <= cgg * P
    jj = np.arange(n)
    offs[jj % P, jj // P] = s * 19
    offs[jj % P, cgg + jj // P] = (NPIX_H + d) * 19
    return offs


def _make_in_maps(inputs_ref, inputs_other, per_core, cgg):
    ref_flat = inputs_ref.reshape(B, C, HW)
    other_flat = inputs_other.reshape(B, C, HW)
    other_cache = {}
    zrow = np.zeros((1, C), dtype=np.float16)
    in_maps = []
    for pc in per_core:
        b, h = pc["b"], pc["h"]
        ref_td = np.ascontiguousarray(
            ref_flat[b, :, h * NPIX_H:(h + 1) * NPIX_H].T).astype(np.float16)
        if b not in other_cache:
            other_cache[b] = np.ascontiguousarray(
                other_flat[b, :, :NPIX].T).astype(np.float16)
        cat = np.concatenate([ref_td, other_cache[b], zrow], axis=0)
        in_maps.append({
            "cat_t": cat,
            "offs": _pack_offs(pc, cgg),
        })
    return in_maps


def kernel(inputs_ref, inputs_other, inds_ref, inds_other, weights):
    from concourse.bass_utils import run_bass_kernel_spmd

    inputs_ref = np.asarray(inputs_ref, dtype=np.float32)
    inputs_other = np.asarray(inputs_other, dtype=np.float32)

    per_core, count = _host_prep(inds_ref, inds_other)
    # exact-fit capacity: compile (and cache) the program for the actual
    # worst-core correspondence count, rounded up to whole 128-columns
    max_n = max(len(pc["s"]) for pc in per_core)
    cgg = max(1, -(-max_n // P))
    nc = _get_program(cgg)

    in_maps = _make_in_maps(inputs_ref, inputs_other, per_core, cgg)
    res = run_bass_kernel_spmd(nc, in_maps, core_ids=list(range(NCORES)))
    total = 0.0
    ln19 = float(np.log(np.float32(19.0)))
    for pc, r in zip(per_core, res.results):
        o = np.asarray(r["out"], dtype=np.float64)
        n_pad = cgg * P - len(pc["s"])
        total += o.sum() + n_pad * ln19
    loss = -total / max(count, 1)
    return np.float32(loss)


# revision 18
# speedup vs baseline: 1.1031x; 1.0872x over previous
"""Trainium2 Bass kernel for nn_CorrClassLoss.

Reference computation (B=4, C=19, H=512, W=1024, N=5000, IGNORE=255):
  ref_class = argmax_c inputs_ref[b].reshape(C, H*W)      # flat W-major
  lin_ref   = 512*y_ref + x_ref    (NOTE: linearized with H, kept faithfully)
  lin_other = 512*y_other + x_other
  gathered  = ref_class[b, lin_ref]
  target[b, lin_other] = gathered  (scatter, last write wins; rest IGNORE)
  loss = mean over non-ignored pixels of -log_softmax(inputs_other)[b, target, px]

Since lin = 512*y + x with x,y in [0,512), only flat positions [0, 262144)
are ever touched, and at most N unique scatter destinations per batch
contribute to the loss:

  loss = -(1/cnt) * sum over unique dests d (last writer j, src s_j) of
         [ x_other[b, cls(s_j), d] - ln(sum_c exp(x_other[b, c, d])) ]
  cls(s) = argmax_c x_ref[b, c, s],  cnt = total unique dests.

Strategy (8 cores, data-parallel over (batch, half-of-correspondences)):
  Host does index-only math (dedup last-wins, split j by the pixel-half of
  s_j, pack padded gather-offset tables) and hands each core a single
  pixel-major fp16 tensor cat_t = [ref_half_t; other_t; zero-row] (a
  layout/sharding choice; all value compute happens on device).
  Device per core: ONE indirect gather (multi-column offset table read
  straight from DRAM) fetches the ref vector at s_j and the other vector
  at d_j for every correspondence; pad slots point at the zero row so no
  masking/memset is needed.  Argmax one-hot via grouped max + is_ge;
  t1 = onehot . other_vec;  t2 = ln(sum_c exp(other_vec[c])).
  Output [P, 1] = per-partition sums of (t1 - t2); host sums partitions,
  adds back the pads' exactly-known -ln(19) contribution, and divides.
"""

import sys

if "/opt/trn_rl_repo" not in sys.path:
    sys.path.insert(0, "/opt/trn_rl_repo")

import numpy as np

B, C, H, W = 4, 19, 512, 1024
HW = H * W                 # 524288
NPIX = 262144              # touched flat range [0, 262144)
NPIX_H = NPIX // 2         # 131072 source pixels per core
N = 5000
NCORES = 8

P = 128                    # partitions
M = NPIX_H + NPIX + 1      # cat_t rows: ref half + other + one zero row
ZERO_ROW = NPIX_H + NPIX          # row index of the zero row

_programs = {}


def _build_program(cgg):
    import concourse.bass as bass
    import concourse.bacc as bacc
    import concourse.mybir as mybir

    GW = cgg * 19

    nc = bacc.Bacc("TRN2", target_bir_lowering=False, debug=False,
                   num_devices=NCORES)

    # fp16 pixel-major shards: [ref half (NPIX_H); other (NPIX); zeros (1)]
    cat_t = nc.dram_tensor("cat_t", [M, C], mybir.dt.float16,
                           kind="ExternalInput")
    # gather offsets (element offsets into cat_t flat): cols [0,cgg) =
    # s_local*19, cols [cgg,2cgg) = (NPIX_H+d)*19; element j at
    # [j%P, j//P]; pads -> ZERO_ROW*19
    offs = nc.dram_tensor("offs", [P, 2 * cgg], mybir.dt.int32,
                          kind="ExternalInput")
    out = nc.dram_tensor("out", [P, cgg], mybir.dt.float32,
                         kind="ExternalOutput")

    cat_flat = cat_t.rearrange("p c -> (p c)")

    f16, f32 = mybir.dt.float16, mybir.dt.float32
    so = nc.alloc_sbuf_tensor("so", [P, 2 * cgg], mybir.dt.int32)
    G = nc.alloc_sbuf_tensor("G", [P, 2 * GW], f16)
    m2 = nc.alloc_sbuf_tensor("m2", [P, cgg], f16)
    eq = nc.alloc_sbuf_tensor("eq", [P, GW], f16)
    e2 = nc.alloc_sbuf_tensor("e2", [P, GW], f16)
    S2 = nc.alloc_sbuf_tensor("S2", [P, cgg], f16)
    L2 = nc.alloc_sbuf_tensor("L2", [P, cgg], f32)
    t1 = nc.alloc_sbuf_tensor("t1", [P, cgg], f16)
    res = nc.alloc_sbuf_tensor("res", [P, cgg], f32)

    Rv = G[:, 0:GW].rearrange("p (g c) -> p g c", c=19)
    R2 = G[:, GW:2 * GW]
    eqv = eq[:].rearrange("p (g c) -> p g c", c=19)
    e2v = e2[:].rearrange("p (g c) -> p g c", c=19)

    sem_so = nc.alloc_semaphore("sem_so")
    sem_g = nc.alloc_semaphore("sem_g")
    sem_e2 = nc.alloc_semaphore("sem_e2")
    sem_s2 = nc.alloc_semaphore("sem_s2")
    sem_l2 = nc.alloc_semaphore("sem_l2")
    sem_res = nc.alloc_semaphore("sem_res")
    sem_out = nc.alloc_semaphore("sem_out")
    dve = nc.alloc_semaphore("dve_chain")

    X = mybir.AxisListType.X
    Op = mybir.AluOpType
    Act = mybir.ActivationFunctionType

    with nc.allow_low_precision(
            reason="fp16 group sums of <=19 values; loss tolerance 2e-2"):
        with nc.Block("k") as block:

            @block.sync
            def _(sync):
                # offset table must live in SBUF for the HW descriptor
                # generator; issue at t=0, no start barrier needed
                sync.dma_start(so[:], offs[:, :]).then_inc(sem_so, 16)
                sync.wait_ge(sem_res, 1)
                sync.dma_start(out[:, :], res[:]).then_inc(sem_out, 16)

            @block.gpsimd
            def _(g):
                g.wait_ge(sem_so, 16)
                # one gather for everything: ref vectors land in G[:, :GW],
                # other vectors in G[:, GW:]; pad slots read the zero row.
                # in_ is the flat view with a leading singleton (one
                # contiguous run) so each partition's 2*GW-element row is
                # one modeled descriptor.
                g.indirect_dma_start(
                    out=G[:],
                    out_offset=None,
                    in_=cat_flat[None, :],
                    in_offset=bass.IndirectOffsetOnAxis(ap=so[:, :], axis=1),
                    bounds_check=None,
                ).then_inc(sem_g, 16)

            @block.scalar
            def _(s):
                s.wait_ge(sem_g, 16)
                s.activation(e2[:], R2, Act.Exp).then_inc(sem_e2, 1)
                s.wait_ge(sem_s2, 1)
                s.activation(L2[:], S2[:], Act.Ln).then_inc(sem_l2, 1)

            @block.vector
            def _(v):
                # explicit chain sems: the engine is in-order, but the race
                # detector (and the SEQ wait-queue bypass) require attached
                # waits; transitive happens-before covers the gather sem
                v.wait_ge(sem_g, 16)
                v.tensor_reduce(out=m2[:], in_=Rv, axis=X,
                                op=Op.max).then_inc(dve, 1)
                v.wait_ge(dve, 1)
                v.tensor_tensor(
                    out=eqv, in0=Rv,
                    in1=m2[:, :, None].to_broadcast([P, cgg, 19]),
                    op=Op.is_ge).then_inc(dve, 1)
                v.wait_ge(dve, 2)
                v.tensor_tensor(out=eq[:], in0=eq[:], in1=R2,
                                op=Op.mult).then_inc(dve, 1)
                # S2 before t1: the final subtract is gated by L2 = Ln(S2)
                v.wait_ge(sem_e2, 1)
                v.tensor_reduce(out=S2[:], in_=e2v, axis=X,
                                op=Op.add).then_inc(sem_s2, 1)
                v.wait_ge(dve, 3)
                v.tensor_reduce(out=t1[:], in_=eqv, axis=X,
                                op=Op.add).then_inc(dve, 1)
                v.wait_ge(sem_l2, 1)
                v.wait_ge(dve, 4)
                v.tensor_tensor(out=res[:], in0=t1[:], in1=L2[:],
                                op=Op.subtract).then_inc(sem_res, 1)

    nc.finalize()
    return nc


def _get_program(cgg):
    if cgg not in _programs:
        _programs[cgg] = _build_program(cgg)
    return _programs[cgg]


def _host_prep(inds_ref, inds_other):
    """Index-only host math: dedup scatter (last wins), partition per core."""
    ir = np.asarray(inds_ref).astype(np.int64)      # [B, 2, N]
    io = np.asarray(inds_other).astype(np.int64)
    valid = ((ir[:, 0] >= 0) & (ir[:, 0] < W) & (ir[:, 1] >= 0) & (ir[:, 1] < H)
             & (io[:, 0] >= 0) & (io[:, 0] < W) & (io[:, 1] >= 0)
             & (io[:, 1] < H))                       # [B, N]
    lin_ref = H * ir[:, 1] + ir[:, 0]                # [B, N]
    lin_other = H * io[:, 1] + io[:, 0]

    per_core = []
    count = 0
    for b in range(B):
        v = valid[b]
        lo = lin_other[b][v]
        lr = np.clip(lin_ref[b][v], 0, HW - 1)
        # last-write-wins dedup on destinations
        u, first_rev = np.unique(lo[::-1], return_index=True)
        last_idx = len(lo) - 1 - first_rev
        d_arr = u.astype(np.int64)
        s_arr = lr[last_idx].astype(np.int64)
        count += len(u)
        for h in range(2):
            sel = (s_arr // NPIX_H) == h
            s_local = s_arr[sel] - h * NPIX_H
            d_sel = d_arr[sel]
            per_core.append({
                "b": b, "h": h,
                "s": s_local, "d": d_sel,
            })
    return per_core, count


def _pack_offs(pc, cgg):
    offs = np.full((P, 2 * cgg), ZERO_ROW * 19, dtype=np.int32)
    s, d = pc["s"], pc["d"]
    n = len(s)
    assert n <= cgg * P
    jj = np.arange(n)
    offs[jj % P, jj // P] = s * 19
    offs[jj % P, cgg + jj // P] = (NPIX_H + d) * 19
    return offs


def _make_in_maps(inputs_ref, inputs_other, per_core, cgg):
    ref_flat = inputs_ref.reshape(B, C, HW)
    other_flat = inputs_other.reshape(B, C, HW)
    other_cache = {}
    zrow = np.zeros((1, C), dtype=np.float16)
    in_maps = []
    for pc in per_core:
        b, h = pc["b"], pc["h"]
        ref_td = np.ascontiguousarray(
            ref_flat[b, :, h * NPIX_H:(h + 1) * NPIX_H].T).astype(np.float16)
        if b not in other_cache:
            other_cache[b] = np.ascontiguousarray(
                other_flat[b, :, :NPIX].T).astype(np.float16)
        cat = np.concatenate([ref_td, other_cache[b], zrow], axis=0)
        in_maps.append({
            "cat_t": cat,
            "offs": _pack_offs(pc, cgg),
        })
    return in_maps


def kernel(inputs_ref, inputs_other, inds_ref, inds_other, weights):
    from concourse.bass_utils import run_bass_kernel_spmd

    inputs_ref = np.asarray(inputs_ref, dtype=np.float32)
    inputs_other = np.asarray(inputs_other, dtype=np.float32)

    per_core, count = _host_prep(inds_ref, inds_other)
    # exact-fit capacity: compile (and cache) the program for the actual
    # worst-core correspondence count, rounded up to whole 128-columns
    max_n = max(len(pc["s"]) for pc in per_core)
    cgg = max(1, -(-max_n // P))
    nc = _get_program(cgg)

    in_maps = _make_in_maps(inputs_ref, inputs_other, per_core, cgg)
    res = run_bass_kernel_spmd(nc, in_maps, core_ids=list(range(NCORES)))
    total = 0.0
    ln19 = float(np.log(np.float32(19.0)))
    for pc, r in zip(per_core, res.results):
        o = np.asarray(r["out"], dtype=np.float64)
        n_pad = cgg * P - len(pc["s"])
        total += o.sum() + n_pad * ln19
    loss = -total / max(count, 1)
    return np.float32(loss)


# revision 25
# speedup vs baseline: 1.1463x; 1.0392x over previous
"""Trainium2 Bass kernel for nn_CorrClassLoss.

Reference computation (B=4, C=19, H=512, W=1024, N=5000, IGNORE=255):
  ref_class = argmax_c inputs_ref[b].reshape(C, H*W)      # flat W-major
  lin_ref   = 512*y_ref + x_ref    (NOTE: linearized with H, kept faithfully)
  lin_other = 512*y_other + x_other
  gathered  = ref_class[b, lin_ref]
  target[b, lin_other] = gathered  (scatter, last write wins; rest IGNORE)
  loss = mean over non-ignored pixels of -log_softmax(inputs_other)[b, target, px]

Since lin = 512*y + x with x,y in [0,512), only flat positions [0, 262144)
are ever touched, and at most N unique scatter destinations per batch
contribute to the loss:

  loss = -(1/cnt) * sum over unique dests d (last writer j, src s_j) of
         [ x_other[b, cls(s_j), d] - ln(sum_c exp(x_other[b, c, d])) ]
  cls(s) = argmax_c x_ref[b, c, s],  cnt = total unique dests.

Strategy (8 cores, data-parallel over (batch, half-of-correspondences)):
  Host does index-only math (dedup last-wins, split j by the pixel-half of
  s_j, pack padded gather-offset tables) and hands each core a single
  pixel-major fp16 tensor cat_t = [ref_half_t; other_t; zero-row] (a
  layout/sharding choice; all value compute happens on device).
  Device per core: ONE indirect gather (multi-column offset table read
  straight from DRAM) fetches the ref vector at s_j and the other vector
  at d_j for every correspondence; pad slots point at the zero row so no
  masking/memset is needed.  Argmax one-hot via grouped max + is_ge;
  t1 = onehot . other_vec;  t2 = ln(sum_c exp(other_vec[c])).
  Output [P, 1] = per-partition sums of (t1 - t2); host sums partitions,
  adds back the pads' exactly-known -ln(19) contribution, and divides.
"""

import sys

if "/opt/trn_rl_repo" not in sys.path:
    sys.path.insert(0, "/opt/trn_rl_repo")

import numpy as np

B, C, H, W = 4, 19, 512, 1024
HW = H * W                 # 524288
NPIX = 262144              # touched flat range [0, 262144)
NPIX_H = NPIX // 2         # 131072 source pixels per core
N = 5000
NCORES = 8

P = 128                    # partitions
M = NPIX_H + NPIX + 1      # cat_t rows: ref half + other + one zero row
ZERO_ROW = NPIX_H + NPIX          # row index of the zero row

_programs = {}


def _build_program(cgg, dev=False):
    import concourse.bass as bass
    import concourse.bacc as bacc
    import concourse.mybir as mybir

    GW = cgg * 19

    # skip the constructor's initial all-engine barrier: nothing in this
    # program reads the built-in const tensors it protects (activations get
    # an explicitly-synchronized zero-bias tensor instead), so every engine
    # can start immediately
    _orig_barrier = bass.Bass.all_engine_barrier
    bass.Bass.all_engine_barrier = lambda self, **kw: None
    try:
        nc = bacc.Bacc("TRN2", target_bir_lowering=False, debug=False,
                       num_devices=NCORES)
    finally:
        bass.Bass.all_engine_barrier = _orig_barrier

    # fp16 pixel-major shards: [ref half (NPIX_H); other (NPIX); zeros (1)]
    cat_t = nc.dram_tensor("cat_t", [M, C], mybir.dt.float16,
                           kind="ExternalInput")
    # gather offsets (element offsets into cat_t flat): cols [0,cgg) =
    # s_local*19, cols [cgg,2cgg) = (NPIX_H+d)*19; element j at
    # [j%P, j//P]; pads -> ZERO_ROW*19
    offs = nc.dram_tensor("offs", [P, 2 * cgg], mybir.dt.int32,
                          kind="ExternalInput")
    out = nc.dram_tensor("out", [P, cgg], mybir.dt.float32,
                         kind="ExternalOutput")

    cat_flat = cat_t.rearrange("p c -> (p c)")

    f16, f32 = mybir.dt.float16, mybir.dt.float32
    so = nc.alloc_sbuf_tensor("so", [P, 2 * cgg], mybir.dt.int32)
    G = nc.alloc_sbuf_tensor("G", [P, 2 * GW], f16)
    m2 = nc.alloc_sbuf_tensor("m2", [P, cgg], f16)
    eq = nc.alloc_sbuf_tensor("eq", [P, GW], f16)
    e2 = nc.alloc_sbuf_tensor("e2", [P, GW], f16)
    S2 = nc.alloc_sbuf_tensor("S2", [P, cgg], f16)
    L2 = nc.alloc_sbuf_tensor("L2", [P, cgg], f32)
    t1 = nc.alloc_sbuf_tensor("t1", [P, cgg], f16)
    res = nc.alloc_sbuf_tensor("res", [P, cgg], f32)
    zb = nc.alloc_sbuf_tensor("zb", [P, 1], f32)

    Rv = G[:, 0:GW].rearrange("p (g c) -> p g c", c=19)
    R2 = G[:, GW:2 * GW]
    eqv = eq[:].rearrange("p (g c) -> p g c", c=19)
    e2v = e2[:].rearrange("p (g c) -> p g c", c=19)

    sem_so = nc.alloc_semaphore("sem_so")
    sem_g = nc.alloc_semaphore("sem_g")
    sem_e2 = nc.alloc_semaphore("sem_e2")
    sem_s2 = nc.alloc_semaphore("sem_s2")
    sem_l2 = nc.alloc_semaphore("sem_l2")
    sem_res = nc.alloc_semaphore("sem_res")
    sem_out = nc.alloc_semaphore("sem_out")
    sem_zb = nc.alloc_semaphore("sem_zb")
    dve = nc.alloc_semaphore("dve_chain")

    X = mybir.AxisListType.X
    Op = mybir.AluOpType
    Act = mybir.ActivationFunctionType

    with nc.allow_low_precision(
            reason="fp16 group sums of <=19 values; loss tolerance 2e-2"):
        with nc.Block("k") as block:

            @block.sync
            def _(sync):
                # offset table must live in SBUF for the HW descriptor
                # generator; issue at t=0, no start barrier needed
                sync.dma_start(so[:], offs[:, :]).then_inc(sem_so, 16)
                sync.wait_ge(sem_res, 1)
                od = sync.dma_start(out[:, :], res[:])
                if dev:
                    # completion sem only needed to satisfy the CoreSim race
                    # detector; on HW the end-of-block engine drain already
                    # guarantees the write landed before kernel exit
                    od.then_inc(sem_out, 16)

            @block.gpsimd
            def _(g):
                g.wait_ge(sem_so, 16)
                # one gather for everything: ref vectors land in G[:, :GW],
                # other vectors in G[:, GW:]; pad slots read the zero row.
                # in_ is the flat view with a leading singleton (one
                # contiguous run) so each partition's 2*GW-element row is
                # one modeled descriptor.
                g.indirect_dma_start(
                    out=G[:],
                    out_offset=None,
                    in_=cat_flat[None, :],
                    in_offset=bass.IndirectOffsetOnAxis(ap=so[:, :], axis=1),
                    bounds_check=None,
                ).then_inc(sem_g, 16)

            @block.scalar
            def _(s):
                s.wait_ge(sem_zb, 1)
                s.wait_ge(sem_g, 16)
                s.activation(e2[:], R2, Act.Exp,
                             bias=zb[:, :]).then_inc(sem_e2, 1)
                s.wait_ge(sem_s2, 1)
                s.activation(L2[:], S2[:], Act.Ln,
                             bias=zb[:, :]).then_inc(sem_l2, 1)

            @block.vector
            def _(v):
                # explicit chain sems: the engine is in-order, but the race
                # detector (and the SEQ wait-queue bypass) require attached
                # waits; transitive happens-before covers the gather sem
                v.memset(zb[:], 0.0).then_inc(sem_zb, 1)
                v.wait_ge(sem_g, 16)
                v.tensor_reduce(out=m2[:], in_=Rv, axis=X,
                                op=Op.max).then_inc(dve, 1)
                v.wait_ge(dve, 1)
                v.tensor_tensor(
                    out=eqv, in0=Rv,
                    in1=m2[:, :, None].to_broadcast([P, cgg, 19]),
                    op=Op.is_ge).then_inc(dve, 1)
                v.wait_ge(dve, 2)
                v.tensor_tensor(out=eq[:], in0=eq[:], in1=R2,
                                op=Op.mult).then_inc(dve, 1)
                # S2 before t1: the final subtract is gated by L2 = Ln(S2)
                v.wait_ge(sem_e2, 1)
                v.tensor_reduce(out=S2[:], in_=e2v, axis=X,
                                op=Op.add).then_inc(sem_s2, 1)
                v.wait_ge(dve, 3)
                v.tensor_reduce(out=t1[:], in_=eqv, axis=X,
                                op=Op.add).then_inc(dve, 1)
                v.wait_ge(sem_l2, 1)
                v.wait_ge(dve, 4)
                v.tensor_tensor(out=res[:], in0=t1[:], in1=L2[:],
                                op=Op.subtract).then_inc(sem_res, 1)

    nc.finalize()
    return nc


def _get_program(cgg, dev=False):
    key = (cgg, dev)
    if key not in _programs:
        _programs[key] = _build_program(cgg, dev=dev)
    return _programs[key]


def _host_prep(inds_ref, inds_other):
    """Index-only host math: dedup scatter (last wins), partition per core."""
    ir = np.asarray(inds_ref).astype(np.int64)      # [B, 2, N]
    io = np.asarray(inds_other).astype(np.int64)
    valid = ((ir[:, 0] >= 0) & (ir[:, 0] < W) & (ir[:, 1] >= 0) & (ir[:, 1] < H)
             & (io[:, 0] >= 0) & (io[:, 0] < W) & (io[:, 1] >= 0)
             & (io[:, 1] < H))                       # [B, N]
    lin_ref = H * ir[:, 1] + ir[:, 0]                # [B, N]
    lin_other = H * io[:, 1] + io[:, 0]

    per_core = []
    count = 0
    for b in range(B):
        v = valid[b]
        lo = lin_other[b][v]
        lr = np.clip(lin_ref[b][v], 0, HW - 1)
        # last-write-wins dedup on destinations
        u, first_rev = np.unique(lo[::-1], return_index=True)
        last_idx = len(lo) - 1 - first_rev
        d_arr = u.astype(np.int64)
        s_arr = lr[last_idx].astype(np.int64)
        count += len(u)
        for h in range(2):
            sel = (s_arr // NPIX_H) == h
            s_local = s_arr[sel] - h * NPIX_H
            d_sel = d_arr[sel]
            per_core.append({
                "b": b, "h": h,
                "s": s_local, "d": d_sel,
            })
    return per_core, count


def _pack_offs(pc, cgg):
    offs = np.full((P, 2 * cgg), ZERO_ROW * 19, dtype=np.int32)
    s, d = pc["s"], pc["d"]
    n = len(s)
    assert n <= cgg * P
    jj = np.arange(n)
    offs[jj % P, jj // P] = s * 19
    offs[jj % P, cgg + jj // P] = (NPIX_H + d) * 19
    return offs


def _make_in_maps(inputs_ref, inputs_other, per_core, cgg):
    ref_flat = inputs_ref.reshape(B, C, HW)
    other_flat = inputs_other.reshape(B, C, HW)
    other_cache = {}
    zrow = np.zeros((1, C), dtype=np.float16)
    in_maps = []
    for pc in per_core:
        b, h = pc["b"], pc["h"]
        ref_td = np.ascontiguousarray(
            ref_flat[b, :, h * NPIX_H:(h + 1) * NPIX_H].T).astype(np.float16)
        if b not in other_cache:
            other_cache[b] = np.ascontiguousarray(
                other_flat[b, :, :NPIX].T).astype(np.float16)
        cat = np.concatenate([ref_td, other_cache[b], zrow], axis=0)
        in_maps.append({
            "cat_t": cat,
            "offs": _pack_offs(pc, cgg),
        })
    return in_maps


def kernel(inputs_ref, inputs_other, inds_ref, inds_other, weights):
    from concourse.bass_utils import run_bass_kernel_spmd

    inputs_ref = np.asarray(inputs_ref, dtype=np.float32)
    inputs_other = np.asarray(inputs_other, dtype=np.float32)

    per_core, count = _host_prep(inds_ref, inds_other)
    # exact-fit capacity: compile (and cache) the program for the actual
    # worst-core correspondence count, rounded up to whole 128-columns
    max_n = max(len(pc["s"]) for pc in per_core)
    cgg = max(1, -(-max_n // P))
    nc = _get_program(cgg)

    in_maps = _make_in_maps(inputs_ref, inputs_other, per_core, cgg)
    res = run_bass_kernel_spmd(nc, in_maps, core_ids=list(range(NCORES)))
    total = 0.0
    ln19 = float(np.log(np.float32(19.0)))
    for pc, r in zip(per_core, res.results):
        o = np.asarray(r["out"], dtype=np.float64)
        n_pad = cgg * P - len(pc["s"])
        total += o.sum() + n_pad * ln19
    loss = -total / max(count, 1)
    return np.float32(loss)


# revision 27
# speedup vs baseline: 1.2792x; 1.1159x over previous
"""Trainium2 Bass kernel for nn_CorrClassLoss.

Reference computation (B=4, C=19, H=512, W=1024, N=5000, IGNORE=255):
  ref_class = argmax_c inputs_ref[b].reshape(C, H*W)      # flat W-major
  lin_ref   = 512*y_ref + x_ref    (NOTE: linearized with H, kept faithfully)
  lin_other = 512*y_other + x_other
  gathered  = ref_class[b, lin_ref]
  target[b, lin_other] = gathered  (scatter, last write wins; rest IGNORE)
  loss = mean over non-ignored pixels of -log_softmax(inputs_other)[b, target, px]

Since lin = 512*y + x with x,y in [0,512), only flat positions [0, 262144)
are ever touched, and at most N unique scatter destinations per batch
contribute to the loss:

  loss = -(1/cnt) * sum over unique dests d (last writer j, src s_j) of
         [ x_other[b, cls(s_j), d] - ln(sum_c exp(x_other[b, c, d])) ]
  cls(s) = argmax_c x_ref[b, c, s],  cnt = total unique dests.

Strategy (8 cores, data-parallel over (batch, half-of-correspondences)):
  Host does index-only math (dedup last-wins, split j by the pixel-half of
  s_j, pack padded gather-offset tables) and hands each core a single
  pixel-major fp16 tensor cat_t = [ref_half_t; other_t; zero-row] (a
  layout/sharding choice; all value compute happens on device).
  Device per core: ONE indirect gather (multi-column offset table read
  straight from DRAM) fetches the ref vector at s_j and the other vector
  at d_j for every correspondence; pad slots point at the zero row so no
  masking/memset is needed.  Argmax one-hot via grouped max + is_ge;
  t1 = onehot . other_vec;  t2 = ln(sum_c exp(other_vec[c])).
  Output [P, 1] = per-partition sums of (t1 - t2); host sums partitions,
  adds back the pads' exactly-known -ln(19) contribution, and divides.
"""

import sys

if "/opt/trn_rl_repo" not in sys.path:
    sys.path.insert(0, "/opt/trn_rl_repo")

import numpy as np

B, C, H, W = 4, 19, 512, 1024
HW = H * W                 # 524288
NPIX = 262144              # touched flat range [0, 262144)
NPIX_H = NPIX // 2         # 131072 source pixels per core
N = 5000
NCORES = 8

P = 128                    # partitions
M = NPIX_H + NPIX + 1      # cat_t rows: ref half + other + one zero row
ZERO_ROW = NPIX_H + NPIX          # row index of the zero row

_programs = {}


def _build_program(cgg, dev=False):
    import concourse.bass as bass
    import concourse.bacc as bacc
    import concourse.mybir as mybir

    GW = cgg * 19

    # skip the constructor's initial all-engine barrier: nothing in this
    # program reads the built-in const tensors it protects (activations get
    # an explicitly-synchronized zero-bias tensor instead), so every engine
    # can start immediately
    _orig_barrier = bass.Bass.all_engine_barrier
    bass.Bass.all_engine_barrier = lambda self, **kw: None
    try:
        nc = bacc.Bacc("TRN2", target_bir_lowering=False, debug=False,
                       num_devices=NCORES)
    finally:
        bass.Bass.all_engine_barrier = _orig_barrier

    # fp16 pixel-major shards: [ref half (NPIX_H); other (NPIX); zeros (1)]
    cat_t = nc.dram_tensor("cat_t", [M, C], mybir.dt.float16,
                           kind="ExternalInput")
    # gather offsets (element offsets into cat_t flat): cols [0,cgg) =
    # s_local*19, cols [cgg,2cgg) = (NPIX_H+d)*19; element j at
    # [j%P, j//P]; pads -> ZERO_ROW*19
    offs = nc.dram_tensor("offs", [P, 2 * cgg], mybir.dt.int32,
                          kind="ExternalInput")
    out = nc.dram_tensor("out", [P, cgg], mybir.dt.float32,
                         kind="ExternalOutput")

    cat_flat = cat_t.rearrange("p c -> (p c)")

    f16, f32 = mybir.dt.float16, mybir.dt.float32
    so = nc.alloc_sbuf_tensor("so", [P, 2 * cgg], mybir.dt.int32)
    G = nc.alloc_sbuf_tensor("G", [P, 2 * GW], f16)
    m2 = nc.alloc_sbuf_tensor("m2", [P, cgg], f16)
    eq = nc.alloc_sbuf_tensor("eq", [P, GW], f16)
    e2 = nc.alloc_sbuf_tensor("e2", [P, GW], f16)
    S2 = nc.alloc_sbuf_tensor("S2", [P, cgg], f16)
    L2 = nc.alloc_sbuf_tensor("L2", [P, cgg], f32)
    t1 = nc.alloc_sbuf_tensor("t1", [P, cgg], f16)
    res = nc.alloc_sbuf_tensor("res", [P, cgg], f32)
    zb = nc.alloc_sbuf_tensor("zb", [P, 1], f32)
    zs = nc.alloc_sbuf_tensor("zs", [P, 1], f32)

    Rv = G[:, 0:GW].rearrange("p (g c) -> p g c", c=19)
    R2 = G[:, GW:2 * GW]
    eqv = eq[:].rearrange("p (g c) -> p g c", c=19)
    e2v = e2[:].rearrange("p (g c) -> p g c", c=19)

    sem_so = nc.alloc_semaphore("sem_so")
    sem_g = nc.alloc_semaphore("sem_g")
    sem_e2 = nc.alloc_semaphore("sem_e2")
    sem_s2 = nc.alloc_semaphore("sem_s2")
    sem_l2 = nc.alloc_semaphore("sem_l2")
    sem_res = nc.alloc_semaphore("sem_res")
    sem_out = nc.alloc_semaphore("sem_out")
    sem_zb = nc.alloc_semaphore("sem_zb")
    dve = nc.alloc_semaphore("dve_chain")

    X = mybir.AxisListType.X
    Op = mybir.AluOpType
    Act = mybir.ActivationFunctionType

    with nc.allow_low_precision(
            reason="fp16 group sums of <=19 values; loss tolerance 2e-2"):
        with nc.Block("k") as block:

            @block.sync
            def _(sync):
                # offset table must live in SBUF for the HW descriptor
                # generator; issue at t=0, no start barrier needed
                sync.dma_start(so[:], offs[:, :]).then_inc(sem_so, 16)
                sync.wait_ge(sem_res, 1)
                od = sync.dma_start(out[:, :], res[:])
                if dev:
                    # completion sem only needed to satisfy the CoreSim race
                    # detector; on HW the end-of-block engine drain already
                    # guarantees the write landed before kernel exit
                    od.then_inc(sem_out, 16)

            @block.gpsimd
            def _(g):
                g.wait_ge(sem_so, 16)
                # one gather for everything: ref vectors land in G[:, :GW],
                # other vectors in G[:, GW:]; pad slots read the zero row.
                # in_ is the flat view with a leading singleton (one
                # contiguous run) so each partition's 2*GW-element row is
                # one modeled descriptor.
                g.indirect_dma_start(
                    out=G[:],
                    out_offset=None,
                    in_=cat_flat[None, :],
                    in_offset=bass.IndirectOffsetOnAxis(ap=so[:, :], axis=1),
                    bounds_check=None,
                ).then_inc(sem_g, 16)

            @block.scalar
            def _(s):
                # dummy activation pulls the Exp table load into the idle
                # window before the gather lands
                s.wait_ge(sem_zb, 1)
                s.activation(zs[:], zb[:, :], Act.Exp, bias=zb[:, :])
                s.wait_ge(sem_g, 16)
                s.activation(e2[:], R2, Act.Exp,
                             bias=zb[:, :]).then_inc(sem_e2, 1)
                s.wait_ge(sem_s2, 1)
                s.activation(L2[:], S2[:], Act.Ln,
                             bias=zb[:, :]).then_inc(sem_l2, 1)

            @block.vector
            def _(v):
                # explicit chain sems: the engine is in-order, but the race
                # detector (and the SEQ wait-queue bypass) require attached
                # waits; transitive happens-before covers the gather sem
                v.memset(zb[:], 0.0).then_inc(sem_zb, 1)
                v.wait_ge(sem_g, 16)
                v.tensor_reduce(out=m2[:], in_=Rv, axis=X,
                                op=Op.max).then_inc(dve, 1)
                v.wait_ge(dve, 1)
                v.tensor_tensor(
                    out=eqv, in0=Rv,
                    in1=m2[:, :, None].to_broadcast([P, cgg, 19]),
                    op=Op.is_ge).then_inc(dve, 1)
                v.wait_ge(dve, 2)
                v.tensor_tensor(out=eq[:], in0=eq[:], in1=R2,
                                op=Op.mult).then_inc(dve, 1)
                # S2 before t1: the final subtract is gated by L2 = Ln(S2)
                v.wait_ge(sem_e2, 1)
                v.tensor_reduce(out=S2[:], in_=e2v, axis=X,
                                op=Op.add).then_inc(sem_s2, 1)
                v.wait_ge(dve, 3)
                v.tensor_reduce(out=t1[:], in_=eqv, axis=X,
                                op=Op.add).then_inc(dve, 1)
                v.wait_ge(sem_l2, 1)
                v.wait_ge(dve, 4)
                v.tensor_tensor(out=res[:], in0=t1[:], in1=L2[:],
                                op=Op.subtract).then_inc(sem_res, 1)

    nc.finalize()
    return nc


def _get_program(cgg, dev=False):
    key = (cgg, dev)
    if key not in _programs:
        _programs[key] = _build_program(cgg, dev=dev)
    return _programs[key]


def _host_prep(inds_ref, inds_other):
    """Index-only host math: dedup scatter (last wins), partition per core."""
    ir = np.asarray(inds_ref).astype(np.int64)      # [B, 2, N]
    io = np.asarray(inds_other).astype(np.int64)
    valid = ((ir[:, 0] >= 0) & (ir[:, 0] < W) & (ir[:, 1] >= 0) & (ir[:, 1] < H)
             & (io[:, 0] >= 0) & (io[:, 0] < W) & (io[:, 1] >= 0)
             & (io[:, 1] < H))                       # [B, N]
    lin_ref = H * ir[:, 1] + ir[:, 0]                # [B, N]
    lin_other = H * io[:, 1] + io[:, 0]

    per_core = []
    count = 0
    for b in range(B):
        v = valid[b]
        lo = lin_other[b][v]
        lr = np.clip(lin_ref[b][v], 0, HW - 1)
        # last-write-wins dedup on destinations
        u, first_rev = np.unique(lo[::-1], return_index=True)
        last_idx = len(lo) - 1 - first_rev
        d_arr = u.astype(np.int64)
        s_arr = lr[last_idx].astype(np.int64)
        count += len(u)
        for h in range(2):
            sel = (s_arr // NPIX_H) == h
            s_local = s_arr[sel] - h * NPIX_H
            d_sel = d_arr[sel]
            per_core.append({
                "b": b, "h": h,
                "s": s_local, "d": d_sel,
            })
    return per_core, count


def _pack_offs(pc, cgg):
    offs = np.full((P, 2 * cgg), ZERO_ROW * 19, dtype=np.int32)
    s, d = pc["s"], pc["d"]
    n = len(s)
    assert n <= cgg * P
    jj = np.arange(n)
    offs[jj % P, jj // P] = s * 19
    offs[jj % P, cgg + jj // P] = (NPIX_H + d) * 19
    return offs


def _make_in_maps(inputs_ref, inputs_other, per_core, cgg):
    ref_flat = inputs_ref.reshape(B, C, HW)
    other_flat = inputs_other.reshape(B, C, HW)
    other_cache = {}
    zrow = np.zeros((1, C), dtype=np.float16)
    in_maps = []
    for pc in per_core:
        b, h = pc["b"], pc["h"]
        ref_td = np.ascontiguousarray(
            ref_flat[b, :, h * NPIX_H:(h + 1) * NPIX_H].T).astype(np.float16)
        if b not in other_cache:
            other_cache[b] = np.ascontiguousarray(
                other_flat[b, :, :NPIX].T).astype(np.float16)
        cat = np.concatenate([ref_td, other_cache[b], zrow], axis=0)
        in_maps.append({
            "cat_t": cat,
            "offs": _pack_offs(pc, cgg),
        })
    return in_maps


def kernel(inputs_ref, inputs_other, inds_ref, inds_other, weights):
    from concourse.bass_utils import run_bass_kernel_spmd

    inputs_ref = np.asarray(inputs_ref, dtype=np.float32)
    inputs_other = np.asarray(inputs_other, dtype=np.float32)

    per_core, count = _host_prep(inds_ref, inds_other)
    # exact-fit capacity: compile (and cache) the program for the actual
    # worst-core correspondence count, rounded up to whole 128-columns
    max_n = max(len(pc["s"]) for pc in per_core)
    cgg = max(1, -(-max_n // P))
    nc = _get_program(cgg)

    in_maps = _make_in_maps(inputs_ref, inputs_other, per_core, cgg)
    res = run_bass_kernel_spmd(nc, in_maps, core_ids=list(range(NCORES)))
    total = 0.0
    ln19 = float(np.log(np.float32(19.0)))
    for pc, r in zip(per_core, res.results):
        o = np.asarray(r["out"], dtype=np.float64)
        n_pad = cgg * P - len(pc["s"])
        total += o.sum() + n_pad * ln19
    loss = -total / max(count, 1)
    return np.float32(loss)


# revision 31
# speedup vs baseline: 1.2895x; 1.0081x over previous
"""Trainium2 Bass kernel for nn_CorrClassLoss.

Reference computation (B=4, C=19, H=512, W=1024, N=5000, IGNORE=255):
  ref_class = argmax_c inputs_ref[b].reshape(C, H*W)      # flat W-major
  lin_ref   = 512*y_ref + x_ref    (NOTE: linearized with H, kept faithfully)
  lin_other = 512*y_other + x_other
  gathered  = ref_class[b, lin_ref]
  target[b, lin_other] = gathered  (scatter, last write wins; rest IGNORE)
  loss = mean over non-ignored pixels of -log_softmax(inputs_other)[b, target, px]

Since lin = 512*y + x with x,y in [0,512), only flat positions [0, 262144)
are ever touched, and at most N unique scatter destinations per batch
contribute to the loss:

  loss = -(1/cnt) * sum over unique dests d (last writer j, src s_j) of
         [ x_other[b, cls(s_j), d] - ln(sum_c exp(x_other[b, c, d])) ]
  cls(s) = argmax_c x_ref[b, c, s],  cnt = total unique dests.

Strategy (8 cores, data-parallel over (batch, half-of-correspondences)):
  Host does index-only math (dedup last-wins, split j by the pixel-half of
  s_j, pack padded gather-offset tables) and hands each core a single
  pixel-major fp16 tensor cat_t = [ref_half_t; other_t; zero-row] (a
  layout/sharding choice; all value compute happens on device).
  Device per core: ONE indirect gather (multi-column offset table read
  straight from DRAM) fetches the ref vector at s_j and the other vector
  at d_j for every correspondence; pad slots point at the zero row so no
  masking/memset is needed.  Argmax one-hot via grouped max + is_ge;
  t1 = onehot . other_vec;  t2 = ln(sum_c exp(other_vec[c])).
  Output [P, 1] = per-partition sums of (t1 - t2); host sums partitions,
  adds back the pads' exactly-known -ln(19) contribution, and divides.
"""

import sys

if "/opt/trn_rl_repo" not in sys.path:
    sys.path.insert(0, "/opt/trn_rl_repo")

import numpy as np

B, C, H, W = 4, 19, 512, 1024
HW = H * W                 # 524288
NPIX = 262144              # touched flat range [0, 262144)
NPIX_H = NPIX // 2         # 131072 source pixels per core
N = 5000
NCORES = 8

P = 128                    # partitions
M = NPIX_H + NPIX + 1      # cat_t rows: ref half + other + one zero row
ZERO_ROW = NPIX_H + NPIX          # row index of the zero row

_programs = {}


def _build_program(cgg, dev=False):
    import concourse.bass as bass
    import concourse.bacc as bacc
    import concourse.mybir as mybir

    GW = cgg * 19

    # skip the constructor's initial all-engine barrier: nothing in this
    # program reads the built-in const tensors it protects (activations get
    # an explicitly-synchronized zero-bias tensor instead), so every engine
    # can start immediately
    _orig_barrier = bass.Bass.all_engine_barrier
    bass.Bass.all_engine_barrier = lambda self, **kw: None
    try:
        nc = bacc.Bacc("TRN2", target_bir_lowering=False, debug=False,
                       num_devices=NCORES)
    finally:
        bass.Bass.all_engine_barrier = _orig_barrier

    # fp16 pixel-major shards: [ref half (NPIX_H); other (NPIX); zeros (1)]
    cat_t = nc.dram_tensor("cat_t", [M, C], mybir.dt.float16,
                           kind="ExternalInput")
    # gather offsets (element offsets into cat_t flat): cols [0,cgg) =
    # s_local*19, cols [cgg,2cgg) = (NPIX_H+d)*19; element j at
    # [j%P, j//P]; pads -> ZERO_ROW*19
    offs = nc.dram_tensor("offs", [P, 2 * cgg], mybir.dt.int32,
                          kind="ExternalInput")
    out = nc.dram_tensor("out", [P, 1], mybir.dt.float32,
                         kind="ExternalOutput")

    cat_flat = cat_t.rearrange("p c -> (p c)")

    f16, f32 = mybir.dt.float16, mybir.dt.float32
    so = nc.alloc_sbuf_tensor("so", [P, 2 * cgg], mybir.dt.int32)
    G = nc.alloc_sbuf_tensor("G", [P, 2 * GW], f16)
    m2 = nc.alloc_sbuf_tensor("m2", [P, cgg], f16)
    eq = nc.alloc_sbuf_tensor("eq", [P, GW], f16)
    e2 = nc.alloc_sbuf_tensor("e2", [P, GW], f16)
    S2 = nc.alloc_sbuf_tensor("S2", [P, cgg], f16)
    L2 = nc.alloc_sbuf_tensor("L2", [P, cgg], f32)
    t1s = nc.alloc_sbuf_tensor("t1s", [P, 1], f32)
    l2s = nc.alloc_sbuf_tensor("l2s", [P, 1], f32)
    res = nc.alloc_sbuf_tensor("res", [P, 1], f32)
    zb = nc.alloc_sbuf_tensor("zb", [P, 1], f32)
    zs = nc.alloc_sbuf_tensor("zs", [P, 1], f32)

    Rv = G[:, 0:GW].rearrange("p (g c) -> p g c", c=19)
    R2 = G[:, GW:2 * GW]
    eqv = eq[:].rearrange("p (g c) -> p g c", c=19)
    e2v = e2[:].rearrange("p (g c) -> p g c", c=19)

    sem_so = nc.alloc_semaphore("sem_so")
    sem_g = nc.alloc_semaphore("sem_g")
    sem_e2 = nc.alloc_semaphore("sem_e2")
    sem_s2 = nc.alloc_semaphore("sem_s2")
    sem_l2 = nc.alloc_semaphore("sem_l2")
    sem_res = nc.alloc_semaphore("sem_res")
    sem_out = nc.alloc_semaphore("sem_out")
    sem_zb = nc.alloc_semaphore("sem_zb")
    dve = nc.alloc_semaphore("dve_chain")

    X = mybir.AxisListType.X
    Op = mybir.AluOpType
    Act = mybir.ActivationFunctionType

    with nc.allow_low_precision(
            reason="fp16 group sums of <=19 values; loss tolerance 2e-2"):
        with nc.Block("k") as block:

            @block.sync
            def _(sync):
                # offset table must live in SBUF for the HW descriptor
                # generator; issue at t=0, no start barrier needed
                sync.dma_start(so[:], offs[:, :]).then_inc(sem_so, 16)
                sync.wait_ge(sem_res, 1)
                od = sync.dma_start(out[:, :], res[:])
                if dev:
                    # completion sem only needed to satisfy the CoreSim race
                    # detector; on HW the end-of-block engine drain already
                    # guarantees the write landed before kernel exit
                    od.then_inc(sem_out, 16)

            @block.gpsimd
            def _(g):
                g.wait_ge(sem_so, 16)
                # one gather for everything: ref vectors land in G[:, :GW],
                # other vectors in G[:, GW:]; pad slots read the zero row.
                # in_ is the flat view with a leading singleton (one
                # contiguous run) so each partition's 2*GW-element row is
                # one modeled descriptor.
                g.indirect_dma_start(
                    out=G[:],
                    out_offset=None,
                    in_=cat_flat[None, :],
                    in_offset=bass.IndirectOffsetOnAxis(ap=so[:, :], axis=1),
                    bounds_check=None,
                ).then_inc(sem_g, 16)

            @block.scalar
            def _(s):
                # dummy activation pulls the Exp table load into the idle
                # window before the gather lands
                s.wait_ge(sem_zb, 1)
                s.activation(zs[:], zb[:, :], Act.Exp, bias=zb[:, :])
                s.wait_ge(sem_g, 16)
                s.activation(e2[:], R2, Act.Exp,
                             bias=zb[:, :]).then_inc(sem_e2, 1)
                s.wait_ge(sem_s2, 1)
                s.activation(L2[:], S2[:], Act.Ln, bias=zb[:, :],
                             accum_out=l2s[:, :]).then_inc(sem_l2, 1)

            @block.vector
            def _(v):
                # explicit chain sems: the engine is in-order, but the race
                # detector (and the SEQ wait-queue bypass) require attached
                # waits; transitive happens-before covers the gather sem
                v.memset(zb[:], 0.0).then_inc(sem_zb, 1)
                v.wait_ge(sem_g, 16)
                v.tensor_reduce(out=m2[:], in_=Rv, axis=X,
                                op=Op.max).then_inc(dve, 1)
                v.wait_ge(dve, 1)
                v.tensor_tensor(
                    out=eqv, in0=Rv,
                    in1=m2[:, :, None].to_broadcast([P, cgg, 19]),
                    op=Op.is_ge).then_inc(dve, 1)
                v.wait_ge(dve, 2)
                # fused: eq *= R2 and t1s = full-row sum(eq*R2) in one op —
                # the host only needs the row sum, never per-group t1
                v.tensor_tensor_reduce(
                    out=eq[:], in0=eq[:], in1=R2, scale=1.0, scalar=0.0,
                    op0=Op.mult, op1=Op.add,
                    accum_out=t1s[:, :]).then_inc(dve, 1)
                v.wait_ge(sem_e2, 1)
                v.tensor_reduce(out=S2[:], in_=e2v, axis=X,
                                op=Op.add).then_inc(sem_s2, 1)
                v.wait_ge(sem_l2, 1)
                v.wait_ge(dve, 3)
                v.tensor_tensor(out=res[:], in0=t1s[:, :], in1=l2s[:, :],
                                op=Op.subtract).then_inc(sem_res, 1)

    nc.finalize()
    return nc


def _get_program(cgg, dev=False):
    key = (cgg, dev)
    if key not in _programs:
        _programs[key] = _build_program(cgg, dev=dev)
    return _programs[key]


def _host_prep(inds_ref, inds_other):
    """Index-only host math: dedup scatter (last wins), partition per core."""
    ir = np.asarray(inds_ref).astype(np.int64)      # [B, 2, N]
    io = np.asarray(inds_other).astype(np.int64)
    valid = ((ir[:, 0] >= 0) & (ir[:, 0] < W) & (ir[:, 1] >= 0) & (ir[:, 1] < H)
             & (io[:, 0] >= 0) & (io[:, 0] < W) & (io[:, 1] >= 0)
             & (io[:, 1] < H))                       # [B, N]
    lin_ref = H * ir[:, 1] + ir[:, 0]                # [B, N]
    lin_other = H * io[:, 1] + io[:, 0]

    per_core = []
    count = 0
    for b in range(B):
        v = valid[b]
        lo = lin_other[b][v]
        lr = np.clip(lin_ref[b][v], 0, HW - 1)
        # last-write-wins dedup on destinations
        u, first_rev = np.unique(lo[::-1], return_index=True)
        last_idx = len(lo) - 1 - first_rev
        d_arr = u.astype(np.int64)
        s_arr = lr[last_idx].astype(np.int64)
        count += len(u)
        for h in range(2):
            sel = (s_arr // NPIX_H) == h
            s_local = s_arr[sel] - h * NPIX_H
            d_sel = d_arr[sel]
            per_core.append({
                "b": b, "h": h,
                "s": s_local, "d": d_sel,
            })
    return per_core, count


def _pack_offs(pc, cgg):
    offs = np.full((P, 2 * cgg), ZERO_ROW * 19, dtype=np.int32)
    s, d = pc["s"], pc["d"]
    n = len(s)
    assert n <= cgg * P
    jj = np.arange(n)
    offs[jj % P, jj // P] = s * 19
    offs[jj % P, cgg + jj // P] = (NPIX_H + d) * 19
    return offs


def _make_in_maps(inputs_ref, inputs_other, per_core, cgg):
    ref_flat = inputs_ref.reshape(B, C, HW)
    other_flat = inputs_other.reshape(B, C, HW)
    other_cache = {}
    zrow = np.zeros((1, C), dtype=np.float16)
    in_maps = []
    for pc in per_core:
        b, h = pc["b"], pc["h"]
        ref_td = np.ascontiguousarray(
            ref_flat[b, :, h * NPIX_H:(h + 1) * NPIX_H].T).astype(np.float16)
        if b not in other_cache:
            other_cache[b] = np.ascontiguousarray(
                other_flat[b, :, :NPIX].T).astype(np.float16)
        cat = np.concatenate([ref_td, other_cache[b], zrow], axis=0)
        in_maps.append({
            "cat_t": cat,
            "offs": _pack_offs(pc, cgg),
        })
    return in_maps


def kernel(inputs_ref, inputs_other, inds_ref, inds_other, weights):
    from concourse.bass_utils import run_bass_kernel_spmd

    inputs_ref = np.asarray(inputs_ref, dtype=np.float32)
    inputs_other = np.asarray(inputs_other, dtype=np.float32)

    per_core, count = _host_prep(inds_ref, inds_other)
    # exact-fit capacity: compile (and cache) the program for the actual
    # worst-core correspondence count, rounded up to whole 128-columns
    max_n = max(len(pc["s"]) for pc in per_core)
    cgg = max(1, -(-max_n // P))
    nc = _get_program(cgg)

    in_maps = _make_in_maps(inputs_ref, inputs_other, per_core, cgg)
    res = run_bass_kernel_spmd(nc, in_maps, core_ids=list(range(NCORES)))
    total = 0.0
    ln19 = float(np.log(np.float32(19.0)))
    for pc, r in zip(per_core, res.results):
        o = np.asarray(r["out"], dtype=np.float64)
        n_pad = cgg * P - len(pc["s"])
        total += o.sum() + n_pad * ln19
    loss = -total / max(count, 1)
    return np.float32(loss)


# revision 32
# speedup vs baseline: 1.3160x; 1.0206x over previous
"""Trainium2 Bass kernel for nn_CorrClassLoss.

Reference computation (B=4, C=19, H=512, W=1024, N=5000, IGNORE=255):
  ref_class = argmax_c inputs_ref[b].reshape(C, H*W)      # flat W-major
  lin_ref   = 512*y_ref + x_ref    (NOTE: linearized with H, kept faithfully)
  lin_other = 512*y_other + x_other
  gathered  = ref_class[b, lin_ref]
  target[b, lin_other] = gathered  (scatter, last write wins; rest IGNORE)
  loss = mean over non-ignored pixels of -log_softmax(inputs_other)[b, target, px]

Since lin = 512*y + x with x,y in [0,512), only flat positions [0, 262144)
are ever touched, and at most N unique scatter destinations per batch
contribute to the loss:

  loss = -(1/cnt) * sum over unique dests d (last writer j, src s_j) of
         [ x_other[b, cls(s_j), d] - ln(sum_c exp(x_other[b, c, d])) ]
  cls(s) = argmax_c x_ref[b, c, s],  cnt = total unique dests.

Strategy (8 cores, data-parallel over (batch, half-of-correspondences)):
  Host does index-only math (dedup last-wins, split j by the pixel-half of
  s_j, pack padded gather-offset tables) and hands each core a single
  pixel-major fp16 tensor cat_t = [ref_half_t; other_t; zero-row] (a
  layout/sharding choice; all value compute happens on device).
  Device per core: ONE indirect gather (multi-column offset table read
  straight from DRAM) fetches the ref vector at s_j and the other vector
  at d_j for every correspondence; pad slots point at the zero row so no
  masking/memset is needed.  Argmax one-hot via grouped max + is_ge;
  t1 = onehot . other_vec;  t2 = ln(sum_c exp(other_vec[c])).
  Output [P, 1] = per-partition sums of (t1 - t2); host sums partitions,
  adds back the pads' exactly-known -ln(19) contribution, and divides.
"""

import sys

if "/opt/trn_rl_repo" not in sys.path:
    sys.path.insert(0, "/opt/trn_rl_repo")

import numpy as np

B, C, H, W = 4, 19, 512, 1024
HW = H * W                 # 524288
NPIX = 262144              # touched flat range [0, 262144)
NPIX_H = NPIX // 2         # 131072 source pixels per core
N = 5000
NCORES = 8

P = 128                    # partitions
M = NPIX_H + NPIX + 1      # cat_t rows: ref half + other + one zero row
ZERO_ROW = NPIX_H + NPIX          # row index of the zero row

_programs = {}


def _build_program(cgg, dev=False):
    import concourse.bass as bass
    import concourse.bacc as bacc
    import concourse.mybir as mybir

    GW = cgg * 19

    # skip the constructor's initial all-engine barrier: nothing in this
    # program reads the built-in const tensors it protects (activations get
    # an explicitly-synchronized zero-bias tensor instead), so every engine
    # can start immediately
    _orig_barrier = bass.Bass.all_engine_barrier
    bass.Bass.all_engine_barrier = lambda self, **kw: None
    try:
        nc = bacc.Bacc("TRN2", target_bir_lowering=False, debug=False,
                       num_devices=NCORES)
    finally:
        bass.Bass.all_engine_barrier = _orig_barrier

    # fp16 pixel-major shards: [ref half (NPIX_H); other (NPIX); zeros (1)]
    cat_t = nc.dram_tensor("cat_t", [M, C], mybir.dt.float16,
                           kind="ExternalInput")
    # gather offsets (element offsets into cat_t flat): cols [0,cgg) =
    # s_local*19, cols [cgg,2cgg) = (NPIX_H+d)*19; element j at
    # [j%P, j//P]; pads -> ZERO_ROW*19
    offs = nc.dram_tensor("offs", [P, 2 * cgg], mybir.dt.int32,
                          kind="ExternalInput")
    out = nc.dram_tensor("out", [P, 1], mybir.dt.float32,
                         kind="ExternalOutput")

    cat_flat = cat_t.rearrange("p c -> (p c)")

    f16, f32 = mybir.dt.float16, mybir.dt.float32
    so = nc.alloc_sbuf_tensor("so", [P, 2 * cgg], mybir.dt.int32)
    G = nc.alloc_sbuf_tensor("G", [P, 2 * GW], f16)
    m2 = nc.alloc_sbuf_tensor("m2", [P, cgg], f16)
    eq = nc.alloc_sbuf_tensor("eq", [P, GW], f16)
    e2 = nc.alloc_sbuf_tensor("e2", [P, GW], f16)
    S2 = nc.alloc_sbuf_tensor("S2", [P, cgg], f16)
    L2 = nc.alloc_sbuf_tensor("L2", [P, cgg], f32)
    t1s = nc.alloc_sbuf_tensor("t1s", [P, 1], f32)
    l2s = nc.alloc_sbuf_tensor("l2s", [P, 1], f32)
    res = nc.alloc_sbuf_tensor("res", [P, 1], f32)
    zb = nc.alloc_sbuf_tensor("zb", [P, 1], f32)
    zs = nc.alloc_sbuf_tensor("zs", [P, 1], f32)

    Rv = G[:, 0:GW].rearrange("p (g c) -> p g c", c=19)
    R2 = G[:, GW:2 * GW]
    eqv = eq[:].rearrange("p (g c) -> p g c", c=19)
    e2v = e2[:].rearrange("p (g c) -> p g c", c=19)

    sem_so = nc.alloc_semaphore("sem_so")
    sem_g = nc.alloc_semaphore("sem_g")
    sem_e2 = nc.alloc_semaphore("sem_e2")
    sem_s2 = nc.alloc_semaphore("sem_s2")
    sem_l2 = nc.alloc_semaphore("sem_l2")
    sem_res = nc.alloc_semaphore("sem_res")
    sem_out = nc.alloc_semaphore("sem_out")
    sem_zb = nc.alloc_semaphore("sem_zb")
    dve = nc.alloc_semaphore("dve_chain")

    X = mybir.AxisListType.X
    Op = mybir.AluOpType
    Act = mybir.ActivationFunctionType

    with nc.allow_low_precision(
            reason="fp16 group sums of <=19 values; loss tolerance 2e-2"):
        with nc.Block("k") as block:

            @block.sync
            def _(sync):
                # offset table must live in SBUF for the HW descriptor
                # generator; issue at t=0, no start barrier needed
                sync.dma_start(so[:], offs[:, :]).then_inc(sem_so, 16)
                sync.wait_ge(sem_res, 1)
                od = sync.dma_start(out[:, :], res[:])
                if dev:
                    # completion sem only needed to satisfy the CoreSim race
                    # detector; on HW the end-of-block engine drain already
                    # guarantees the write landed before kernel exit
                    od.then_inc(sem_out, 16)

            @block.gpsimd
            def _(g):
                g.wait_ge(sem_so, 16)
                # one gather for everything: ref vectors land in G[:, :GW],
                # other vectors in G[:, GW:]; pad slots read the zero row.
                # in_ is the flat view with a leading singleton (one
                # contiguous run) so each partition's 2*GW-element row is
                # one modeled descriptor.
                g.indirect_dma_start(
                    out=G[:],
                    out_offset=None,
                    in_=cat_flat[None, :],
                    in_offset=bass.IndirectOffsetOnAxis(ap=so[:, :], axis=1),
                    bounds_check=None,
                ).then_inc(sem_g, 16)

            @block.scalar
            def _(s):
                # dummy activation pulls the Exp table load into the idle
                # window before the gather lands
                s.wait_ge(sem_zb, 1)
                s.activation(zs[:], zb[:, :], Act.Exp, bias=zb[:, :])
                s.wait_ge(sem_g, 16)
                s.activation(e2[:], R2, Act.Exp,
                             bias=zb[:, :]).then_inc(sem_e2, 1)
                s.wait_ge(sem_s2, 1)
                s.activation(L2[:], S2[:], Act.Ln, bias=zb[:, :],
                             accum_out=l2s[:, :]).then_inc(sem_l2, 1)

            @block.vector
            def _(v):
                # the engine WAIT queue is in-order, so same-engine RAW
                # hazards are safe without sems; the explicit chain sems are
                # only added in dev builds to satisfy the CoreSim race
                # detector (transitive happens-before covers the gather sem)
                v.memset(zb[:], 0.0).then_inc(sem_zb, 1)
                v.wait_ge(sem_g, 16)
                i1 = v.tensor_reduce(out=m2[:], in_=Rv, axis=X, op=Op.max)
                if dev:
                    i1.then_inc(dve, 1)
                    v.wait_ge(dve, 1)
                i2 = v.tensor_tensor(
                    out=eqv, in0=Rv,
                    in1=m2[:, :, None].to_broadcast([P, cgg, 19]),
                    op=Op.is_ge)
                if dev:
                    i2.then_inc(dve, 1)
                    v.wait_ge(dve, 2)
                # fused: eq *= R2 and t1s = full-row sum(eq*R2) in one op —
                # the host only needs the row sum, never per-group t1
                i3 = v.tensor_tensor_reduce(
                    out=eq[:], in0=eq[:], in1=R2, scale=1.0, scalar=0.0,
                    op0=Op.mult, op1=Op.add, accum_out=t1s[:, :])
                if dev:
                    i3.then_inc(dve, 1)
                v.wait_ge(sem_e2, 1)
                v.tensor_reduce(out=S2[:], in_=e2v, axis=X,
                                op=Op.add).then_inc(sem_s2, 1)
                v.wait_ge(sem_l2, 1)
                if dev:
                    v.wait_ge(dve, 3)
                v.tensor_tensor(out=res[:], in0=t1s[:, :], in1=l2s[:, :],
                                op=Op.subtract).then_inc(sem_res, 1)

    nc.finalize()
    return nc


def _get_program(cgg, dev=False):
    key = (cgg, dev)
    if key not in _programs:
        _programs[key] = _build_program(cgg, dev=dev)
    return _programs[key]


def _host_prep(inds_ref, inds_other):
    """Index-only host math: dedup scatter (last wins), partition per core."""
    ir = np.asarray(inds_ref).astype(np.int64)      # [B, 2, N]
    io = np.asarray(inds_other).astype(np.int64)
    valid = ((ir[:, 0] >= 0) & (ir[:, 0] < W) & (ir[:, 1] >= 0) & (ir[:, 1] < H)
             & (io[:, 0] >= 0) & (io[:, 0] < W) & (io[:, 1] >= 0)
             & (io[:, 1] < H))                       # [B, N]
    lin_ref = H * ir[:, 1] + ir[:, 0]                # [B, N]
    lin_other = H * io[:, 1] + io[:, 0]

    per_core = []
    count = 0
    for b in range(B):
        v = valid[b]
        lo = lin_other[b][v]
        lr = np.clip(lin_ref[b][v], 0, HW - 1)
        # last-write-wins dedup on destinations
        u, first_rev = np.unique(lo[::-1], return_index=True)
        last_idx = len(lo) - 1 - first_rev
        d_arr = u.astype(np.int64)
        s_arr = lr[last_idx].astype(np.int64)
        count += len(u)
        for h in range(2):
            sel = (s_arr // NPIX_H) == h
            s_local = s_arr[sel] - h * NPIX_H
            d_sel = d_arr[sel]
            per_core.append({
                "b": b, "h": h,
                "s": s_local, "d": d_sel,
            })
    return per_core, count


def _pack_offs(pc, cgg):
    offs = np.full((P, 2 * cgg), ZERO_ROW * 19, dtype=np.int32)
    s, d = pc["s"], pc["d"]
    n = len(s)
    assert n <= cgg * P
    jj = np.arange(n)
    offs[jj % P, jj // P] = s * 19
    offs[jj % P, cgg + jj // P] = (NPIX_H + d) * 19
    return offs


def _make_in_maps(inputs_ref, inputs_other, per_core, cgg):
    ref_flat = inputs_ref.reshape(B, C, HW)
    other_flat = inputs_other.reshape(B, C, HW)
    other_cache = {}
    zrow = np.zeros((1, C), dtype=np.float16)
    in_maps = []
    for pc in per_core:
        b, h = pc["b"], pc["h"]
        ref_td = np.ascontiguousarray(
            ref_flat[b, :, h * NPIX_H:(h + 1) * NPIX_H].T).astype(np.float16)
        if b not in other_cache:
            other_cache[b] = np.ascontiguousarray(
                other_flat[b, :, :NPIX].T).astype(np.float16)
        cat = np.concatenate([ref_td, other_cache[b], zrow], axis=0)
        in_maps.append({
            "cat_t": cat,
            "offs": _pack_offs(pc, cgg),
        })
    return in_maps


def kernel(inputs_ref, inputs_other, inds_ref, inds_other, weights):
    from concourse.bass_utils import run_bass_kernel_spmd

    inputs_ref = np.asarray(inputs_ref, dtype=np.float32)
    inputs_other = np.asarray(inputs_other, dtype=np.float32)

    per_core, count = _host_prep(inds_ref, inds_other)
    # exact-fit capacity: compile (and cache) the program for the actual
    # worst-core correspondence count, rounded up to whole 128-columns
    max_n = max(len(pc["s"]) for pc in per_core)
    cgg = max(1, -(-max_n // P))
    nc = _get_program(cgg)

    in_maps = _make_in_maps(inputs_ref, inputs_other, per_core, cgg)
    res = run_bass_kernel_spmd(nc, in_maps, core_ids=list(range(NCORES)))
    total = 0.0
    ln19 = float(np.log(np.float32(19.0)))
    for pc, r in zip(per_core, res.results):
        o = np.asarray(r["out"], dtype=np.float64)
        n_pad = cgg * P - len(pc["s"])
        total += o.sum() + n_pad * ln19
    loss = -total / max(count, 1)
    return np.float32(loss)
